# revision 16
# baseline (speedup 1.0000x reference)
"""Trainium2 Bass kernel for nn_LinearTransformer_75892072120460.

Math: the reference returns out[:, 0, 0] -- only sequence position 0 of the
final head survives.  Linear attention at query position 0 collapses to
    s_l   = Q0 . (elu(kraw_l) + 1)          (scalar weight per position)
    attn0 = (sum_l s_l h_l) @ wv.T ... / (sum_l s_l + eps)
with kraw = x @ (w_in.T wk.T) + bc, which is the only O(L) dense work.

Device (per core, 2 batches of the 16):  P = x_aug @ Wc_aug  (K=33 matmul,
bias folded via ones-row), then elu(P) = relu(P) + (min(e^P,1) - 1) computed
as one ACT exp pass + dual-op tensor_scalar + split relu, then the weighted
sum over e as PE matmuls with Q0 broadcast to 33 output rows, and a fused
tensor_tensor_reduce producing xs (32 dims) + ssum per chunk.

Host: weight folding, Q0 at position 0 (16x512), and the tiny [16]-row head.
"""

import os
import numpy as np
import ml_dtypes

N, L, IN_DIM, D, E = 16, 4096, 32, 512, 512
EPS_ATTN = 1e-6
EPS_LN = 1e-5
N_CORES = 8
B_PER_CORE = N // N_CORES          # 2
CHUNK = 512                        # rows (l) per chunk
NCHUNK = L // CHUNK                # 8
NJ = 4                             # e-chunks of 128
SPL = 1024                         # relu split point: [0:SPL] on ACT, rest on DVE

_CACHED = {}
LAST_RESULTS = None


def _build_bass():
    if "nc" in _CACHED:
        return _CACHED["nc"]
    import concourse.bass as bass
    import concourse.tile as tile
    import concourse.mybir as mybir
    from concourse import bacc

    f32 = mybir.dt.float32
    bf16 = mybir.dt.bfloat16
    AF = mybir.ActivationFunctionType
    OP = mybir.AluOpType

    nc = bacc.Bacc(None, target_bir_lowering=False)
    xt = nc.dram_tensor("xt", [B_PER_CORE, 33, L], bf16, kind="ExternalInput")
    wca = nc.dram_tensor("wca", [33, E], bf16, kind="ExternalInput")
    q0r = nc.dram_tensor("q0r", [B_PER_CORE, 128, 33 * NJ], bf16, kind="ExternalInput")
    xss = nc.dram_tensor("xss", [B_PER_CORE, 33, 1], f32, kind="ExternalOutput")

    HALF = 2 * CHUNK  # two e-chunks per PSUM tile

    with tile.TileContext(nc) as tc:
        with (
            tc.tile_pool(name="const", bufs=1) as const,
            tc.tile_pool(name="work", bufs=3) as work,
            tc.tile_pool(name="accp", bufs=1) as accp,
            tc.tile_pool(name="psP", bufs=3, space=bass.MemorySpace.PSUM) as psP,
            tc.tile_pool(name="psS", bufs=2, space=bass.MemorySpace.PSUM) as psS,
        ):
            # Wait-absorbers: several instruction structs (fp32 self-loading
            # LDW, STT) accept only ONE sem wait.  A single-tensor read on
            # each engine advances its observed DMA tick first, so the real
            # compute instructions need at most one wait each.
            wc_sb = const.tile([33, E], bf16, tag="wc")
            nc.gpsimd.dma_start(out=wc_sb[:], in_=wca[:])
            nc.tensor.ldweights(wc_sb[:, 0:128])

            xbs, q0s, slots = [], [], []
            for n in range(B_PER_CORE):
                xb = const.tile([33, L], bf16, tag=f"xb{n}")
                for c in range(NCHUNK):
                    cs = slice(c * CHUNK, (c + 1) * CHUNK)
                    nc.gpsimd.dma_start(out=xb[:, cs], in_=xt[n][:, cs])
                q0b = const.tile([128, 33 * NJ], bf16, tag=f"q0{n}")
                nc.gpsimd.dma_start(out=q0b[:], in_=q0r[n])
                nc.tensor.ldweights(q0b[:, 0:33])
                sl = accp.tile([33, NCHUNK], f32, tag=f"sl{n}")
                xbs.append(xb); q0s.append(q0b); slots.append(sl)

            for n in range(B_PER_CORE):
                xb, q0b, sl = xbs[n], q0s[n], slots[n]
                for c in range(NCHUNK):
                    cs = slice(c * CHUNK, (c + 1) * CHUNK)
                    relu_on_act = (3 * c) % 8 < 3
                    # main matmuls into two PSUM half-spans (2 banks each)
                    Ph = []
                    for h in range(2):
                        P_ps = psP.tile([128, HALF], f32, tag="P")
                        for jj in range(2):
                            j = 2 * h + jj
                            nc.tensor.matmul(
                                P_ps[:, jj * CHUNK:(jj + 1) * CHUNK],
                                wc_sb[:, j * 128:(j + 1) * 128],
                                xb[:, cs],
                                start=True, stop=True,
                            )
                        Ph.append(P_ps)
                    # pass A: E = exp(P)   (ACT, PSUM->SBUF, bf16 out)
                    Eb = work.tile([128, NJ * CHUNK], bf16, tag="E")
                    for h in range(2):
                        nc.scalar.activation(
                            Eb[:, h * HALF:(h + 1) * HALF], Ph[h][:], AF.Exp)
                    # pass B: G = min(E,1) - 1   (DVE 4x bf16, one op)
                    Gb = work.tile([128, NJ * CHUNK], bf16, tag="G")
                    nc.vector.tensor_scalar(
                        Gb[:], Eb[:], 1.0, 1.0, OP.min, OP.subtract)
                    # pass R: T = relu(P), engine chosen per chunk
                    # (per-engine tag: slot reuse stays same-engine WAW)
                    Tb = work.tile([128, NJ * CHUNK], bf16,
                                   tag="Ta" if relu_on_act else "Tv")
                    for h in range(2):
                        dst = Tb[:, h * HALF:(h + 1) * HALF]
                        if relu_on_act:
                            nc.scalar.activation(dst, Ph[h][:], AF.Relu)
                        else:
                            nc.vector.tensor_scalar(
                                dst, Ph[h][:], 0.0, None, OP.max)
                    # dots: sB[33, l] = sum_e Q0_e * (G + T)    (M=33 broadcast)
                    sB = psS.tile([33, CHUNK], f32, tag="sB")
                    for j in range(NJ):
                        nc.tensor.matmul(
                            sB[:], q0b[:, 33 * j:33 * (j + 1)],
                            Gb[:, j * CHUNK:(j + 1) * CHUNK],
                            start=(j == 0), stop=False)
                    for j in range(NJ):
                        nc.tensor.matmul(
                            sB[:], q0b[:, 33 * j:33 * (j + 1)],
                            Tb[:, j * CHUNK:(j + 1) * CHUNK],
                            start=False, stop=(j == NJ - 1))
                    # fused xs/ssum: accum_out[p] = sum_l xb[p, l] * sB[p, l]
                    # (tiny toucher first: absorbs this chunk's DMA tick on
                    # DVE so the STT below only waits on PE)
                    nc.vector.tensor_copy(sl[0:1, c:c + 1], xb[0:1, c * CHUNK:c * CHUNK + 1])
                    junk = work.tile([33, CHUNK], f32, tag="junk")
                    nc.vector.scalar_tensor_tensor(
                        out=junk[:],
                        in0=xb[:, cs],
                        scalar=1.0,
                        in1=sB[:],
                        op0=OP.mult,
                        op1=OP.mult,
                        accum_out=sl[:, c:c + 1],
                    )
                # per-batch: reduce the chunk partials, ship out
                xo = accp.tile([33, 1], f32, tag=f"xo{n}")
                nc.vector.tensor_reduce(
                    out=xo[:], in_=sl[:], axis=mybir.AxisListType.X, op=OP.add)
                nc.sync.dma_start(out=xss[n], in_=xo[:])

    nc.compile()
    _CACHED["nc"] = nc
    return nc


def _elu(x):
    return np.where(x > 0, x, np.expm1(np.minimum(x, 0.0)))


def _ln(x, g, b):
    mu = x.mean(-1, keepdims=True)
    var = ((x - mu) ** 2).mean(-1, keepdims=True)
    return (x - mu) / np.sqrt(var + EPS_LN) * g + b


def kernel(x, w_in, b_in, wq, bq, wk, bk, wv, bv, wo, bo, g1, b1,
           w_ff1, b_ff1, w_ff2, b_ff2, g2, b2, gf, bf, w_fc, b_fc):
    global LAST_RESULTS
    from concourse.bass_utils import run_bass_kernel_spmd

    x = np.asarray(x, np.float32)
    f32 = np.float32

    # ---- host weight folding (params only) ----
    Wc = (w_in.T @ wk.T).astype(f32)                    # [32, 512]
    bc = (b_in @ wk.T + bk).astype(f32)                 # [512]
    wca = np.concatenate([Wc, bc[None, :]], 0)          # [33, 512]

    # ---- Q0 at position 0 (host; 16x512, ~0.5 MFLOP) ----
    x0 = x[:, 0, :]                                     # [16, 32]
    h0 = (x0 @ w_in.T + b_in).astype(f32)               # [16, 512]
    q0 = (_elu(h0 @ wq.T + bq) + 1.0).astype(f32)       # [16, 512]
    q0sum = q0.sum(1)                                   # [16]

    # q0 replicated to 33 cols per e-chunk, partition-major
    q0r = np.zeros((N, 128, 33 * NJ), dtype=f32)
    for j in range(NJ):
        q0r[:, :, 33 * j:33 * (j + 1)] = q0[:, 128 * j:128 * (j + 1)][:, :, None]
    q0r = q0r.astype(ml_dtypes.bfloat16)

    # x^T with ones row (bias + ssum row)
    xt = np.concatenate(
        [np.ascontiguousarray(x.transpose(0, 2, 1)),
         np.ones((N, 1, L), f32)], axis=1)              # [16, 33, 4096]

    nc = _build_bass()
    in_maps = []
    for i in range(N_CORES):
        sl = slice(i * B_PER_CORE, (i + 1) * B_PER_CORE)
        in_maps.append({
            "xt": np.ascontiguousarray(xt[sl]).astype(ml_dtypes.bfloat16),
            "wca": wca.astype(ml_dtypes.bfloat16),
            "q0r": np.ascontiguousarray(q0r[sl]),
        })

    res = run_bass_kernel_spmd(nc, in_maps, core_ids=list(range(N_CORES)))
    LAST_RESULTS = res
    xss_dev = np.concatenate([r["xss"][:, :, 0] for r in res.results], 0)  # [16, 33]

    # ---- host epilogue ([16]-row head) ----
    # true weighted sums: add q0sum * sum_l x_aug_l  (the "+1" of elu+1)
    xsum = np.concatenate([x.sum(1), np.full((N, 1), float(L), f32)], 1)   # [16, 33]
    xss_t = xss_dev + q0sum[:, None] * xsum
    xs, ssum = xss_t[:, :32], xss_t[:, 32]

    Z = 1.0 / (ssum + EPS_ATTN)                         # [16]
    hsum = xs @ w_in.T + ssum[:, None] * b_in           # sum_l s_l h_l
    v_att = hsum @ wv.T + ssum[:, None] * bv            # sum_l s_l v_l
    attn_o = (v_att * Z[:, None]) @ wo.T + bo
    t1 = h0 + attn_o
    h1 = _ln(t1, g1, b1)
    y = np.maximum(h1 @ w_ff1.T + b_ff1, 0.0) @ w_ff2.T + b_ff2
    h2 = _ln(h1 + y, g2, b2)
    h3 = _ln(h2, gf, bf)
    out = h3 @ w_fc.T + b_fc                            # [16, 1]
    return out[:, 0].astype(f32)


# revision 23
# speedup vs baseline: 2.2000x; 2.2000x over previous
"""Trainium2 Bass kernel for nn_LinearTransformer_75892072120460.

Math: the reference returns out[:, 0, 0] -- only sequence position 0 of the
final head survives.  Linear attention at query position 0 collapses to
    s_l   = Q0 . (elu(kraw_l) + 1)          (scalar weight per position)
    attn0 = (sum_l s_l h_l) @ wv.T ... / (sum_l s_l + eps)
with kraw = x @ (w_in.T wk.T) + bc, which is the only O(L) dense work.

Device (per core, 2 batches of the 16):  P = x_aug @ Wc_aug  (K=33 matmul,
bias folded via ones-row), then elu(P) = relu(P) + (min(e^P,1) - 1) computed
as one ACT exp pass + dual-op tensor_scalar + split relu, then the weighted
sum over e as PE matmuls with Q0 broadcast to 33 output rows, and a fused
tensor_tensor_reduce producing xs (32 dims) + ssum per chunk.

Host: weight folding, Q0 at position 0 (16x512), and the tiny [16]-row head.
"""

import os
import numpy as np
import ml_dtypes

N, L, IN_DIM, D, E = 16, 4096, 32, 512, 512
EPS_ATTN = 1e-6
EPS_LN = 1e-5
N_CORES = 8
B_PER_CORE = N // N_CORES          # 2
CHUNK = 512                        # rows (l) per chunk
NCHUNK = L // CHUNK                # 8
NJ = 4                             # e-chunks of 128
SPL = 1024                         # relu split point: [0:SPL] on ACT, rest on DVE

_CACHED = {}
LAST_RESULTS = None


def _build_bass(workbufs=3, relu_mod=(0, 8, 0), psp_bufs=3, pss_bufs=2,
                hwdge=True, early_gdots=False, cache=True):
    if cache and "nc" in _CACHED:
        return _CACHED["nc"]
    import concourse.bass as bass
    import concourse.tile as tile
    import concourse.mybir as mybir
    from concourse import bacc

    f32 = mybir.dt.float32
    bf16 = mybir.dt.bfloat16
    AF = mybir.ActivationFunctionType
    OP = mybir.AluOpType

    nc = bacc.Bacc(None, target_bir_lowering=False)
    xt = nc.dram_tensor("xt", [B_PER_CORE, 33, L], bf16, kind="ExternalInput")
    wca = nc.dram_tensor("wca", [33, E], bf16, kind="ExternalInput")
    q0r = nc.dram_tensor("q0r", [B_PER_CORE, 128, 64 * NJ], bf16, kind="ExternalInput")
    xss = nc.dram_tensor("xss", [B_PER_CORE, 128, 1], f32, kind="ExternalOutput")

    HALF = 2 * CHUNK  # two e-chunks per PSUM tile

    with tile.TileContext(nc) as tc:
        with (
            tc.tile_pool(name="const", bufs=1) as const,
            tc.tile_pool(name="work", bufs=workbufs) as work,
            tc.tile_pool(name="accp", bufs=1) as accp,
            tc.tile_pool(name="psP", bufs=psp_bufs, space=bass.MemorySpace.PSUM) as psP,
            tc.tile_pool(name="psS", bufs=pss_bufs, space=bass.MemorySpace.PSUM) as psS,
        ):
            # Wait-absorbers: several instruction structs (fp32 self-loading
            # LDW, STT) accept only ONE sem wait.  A single-tensor read on
            # each engine advances its observed DMA tick first, so the real
            # compute instructions need at most one wait each.
            wc_sb = const.tile([33, E], bf16, tag="wc")
            nc.gpsimd.dma_start(out=wc_sb[:], in_=wca[:])
            nc.tensor.ldweights(wc_sb[:, 0:64], tile_position=(0, 0))

            xbs, q0s, slots = [], [], []
            for n in range(B_PER_CORE):
                xb = const.tile([128, L], bf16, tag=f"xb{n}")
                for c in range(NCHUNK):
                    cs = slice(c * CHUNK, (c + 1) * CHUNK)
                    eng = nc.sync if hwdge else nc.gpsimd
                    eng.dma_start(out=xb[0:33, cs], in_=xt[n][:, cs])
                    eng.dma_start(out=xb[64:97, cs], in_=xt[n][:, cs])
                q0b = const.tile([128, 64 * NJ], bf16, tag=f"q0{n}")
                nc.gpsimd.dma_start(out=q0b[:], in_=q0r[n])
                nc.tensor.ldweights(q0b[:, 0:64], tile_position=(0, 0))
                sl = accp.tile([128, NCHUNK], f32, tag=f"sl{n}")
                xbs.append(xb); q0s.append(q0b); slots.append(sl)

            for n in range(B_PER_CORE):
                xb, q0b, sl = xbs[n], q0s[n], slots[n]
                for c in range(NCHUNK):
                    cs = slice(c * CHUNK, (c + 1) * CHUNK)
                    relu_on_act = (relu_mod[0] * c) % relu_mod[1] < relu_mod[2]
                    # main matmuls into two PSUM half-spans (2 banks each)
                    Ph = []
                    for h in range(2):
                        P_ps = psP.tile([128, HALF], f32, tag="P")
                        for jj in range(2):
                            j = 2 * h + jj
                            for q in range(2):
                                nc.tensor.matmul(
                                    P_ps[64 * q:64 * (q + 1),
                                         jj * CHUNK:(jj + 1) * CHUNK],
                                    wc_sb[:, j * 128 + 64 * q:j * 128 + 64 * (q + 1)],
                                    xb[0:33, cs],
                                    start=True, stop=True,
                                    tile_position=(0, 64 * q),
                                )
                        Ph.append(P_ps)
                    # pass A: E = exp(P)   (ACT, PSUM->SBUF, bf16 out)
                    Eb = work.tile([128, NJ * CHUNK], bf16, tag="E")
                    for h in range(2):
                        nc.scalar.activation(
                            Eb[:, h * HALF:(h + 1) * HALF], Ph[h][:], AF.Exp)
                    # pass B: G = min(E,1) - 1   (DVE 4x bf16, one op)
                    Gb = work.tile([128, NJ * CHUNK], bf16, tag="G")
                    nc.vector.tensor_scalar(
                        Gb[:], Eb[:], 1.0, 1.0, OP.min, OP.subtract)
                    sB = psS.tile([128, CHUNK], f32, tag="sB")

                    def dot(j, src, start, stop):
                        grp = 0 if j < 2 else 1
                        nc.tensor.matmul(
                            sB[64 * grp:64 * (grp + 1), :],
                            q0b[:, 64 * j:64 * j + 64],
                            src[:, j * CHUNK:(j + 1) * CHUNK],
                            start=start, stop=stop,
                            tile_position=(0, 64 * grp),
                        )

                    def gdots():
                        for j in range(NJ):
                            dot(j, Gb, start=(j in (0, 2)), stop=False)

                    if early_gdots:
                        gdots()
                    # pass R: T = relu(P), engine chosen per chunk
                    # (per-engine tag: slot reuse stays same-engine WAW)
                    Tb = work.tile([128, NJ * CHUNK], bf16,
                                   tag="Ta" if relu_on_act else "Tv")
                    for h in range(2):
                        dst = Tb[:, h * HALF:(h + 1) * HALF]
                        if relu_on_act:
                            nc.scalar.activation(dst, Ph[h][:], AF.Relu)
                        else:
                            nc.vector.tensor_scalar(
                                dst, Ph[h][:], 0.0, None, OP.max)
                    # dots: sB[33, l] = sum_e Q0_e * (G + T)    (M=33 broadcast)
                    if not early_gdots:
                        gdots()
                    for j in range(NJ):
                        dot(j, Tb, start=False, stop=(j in (1, 3)))
                    # fused xs/ssum: accum_out[p] = sum_l xb[p, l] * sB[p, l]
                    # (tiny toucher first: absorbs this chunk's DMA tick on
                    # DVE so the STT below only waits on PE)
                    nc.vector.tensor_copy(sl[0:1, c:c + 1], xb[0:1, c * CHUNK:c * CHUNK + 1])
                    junk = work.tile([128, CHUNK], f32, tag="junk")
                    nc.vector.scalar_tensor_tensor(
                        out=junk[:],
                        in0=xb[:, cs],
                        scalar=1.0,
                        in1=sB[:],
                        op0=OP.mult,
                        op1=OP.mult,
                        accum_out=sl[:, c:c + 1],
                    )
                # per-batch: reduce the chunk partials, ship out
                xo = accp.tile([128, 1], f32, tag=f"xo{n}")
                nc.vector.tensor_reduce(
                    out=xo[:], in_=sl[:], axis=mybir.AxisListType.X, op=OP.add)
                nc.sync.dma_start(out=xss[n], in_=xo[:])

    nc.compile()
    if cache:
        _CACHED["nc"] = nc
    return nc


def _elu(x):
    return np.where(x > 0, x, np.expm1(np.minimum(x, 0.0)))


def _ln(x, g, b):
    mu = x.mean(-1, keepdims=True)
    var = ((x - mu) ** 2).mean(-1, keepdims=True)
    return (x - mu) / np.sqrt(var + EPS_LN) * g + b


def kernel(x, w_in, b_in, wq, bq, wk, bk, wv, bv, wo, bo, g1, b1,
           w_ff1, b_ff1, w_ff2, b_ff2, g2, b2, gf, bf, w_fc, b_fc):
    global LAST_RESULTS
    from concourse.bass_utils import run_bass_kernel_spmd

    x = np.asarray(x, np.float32)
    f32 = np.float32

    # ---- host weight folding (params only) ----
    Wc = (w_in.T @ wk.T).astype(f32)                    # [32, 512]
    bc = (b_in @ wk.T + bk).astype(f32)                 # [512]
    wca = np.concatenate([Wc, bc[None, :]], 0)          # [33, 512]

    # ---- Q0 at position 0 (host; 16x512, ~0.5 MFLOP) ----
    x0 = x[:, 0, :]                                     # [16, 32]
    h0 = (x0 @ w_in.T + b_in).astype(f32)               # [16, 512]
    q0 = (_elu(h0 @ wq.T + bq) + 1.0).astype(f32)       # [16, 512]
    q0sum = q0.sum(1)                                   # [16]

    # q0 replicated to 64 cols per e-chunk, partition-major
    q0r = np.zeros((N, 128, 64 * NJ), dtype=f32)
    for j in range(NJ):
        q0r[:, :, 64 * j:64 * (j + 1)] = q0[:, 128 * j:128 * (j + 1)][:, :, None]
    q0r = q0r.astype(ml_dtypes.bfloat16)

    # x^T with ones row (bias + ssum row)
    xt = np.concatenate(
        [np.ascontiguousarray(x.transpose(0, 2, 1)),
         np.ones((N, 1, L), f32)], axis=1)              # [16, 33, 4096]

    nc = _build_bass()
    in_maps = []
    for i in range(N_CORES):
        sl = slice(i * B_PER_CORE, (i + 1) * B_PER_CORE)
        in_maps.append({
            "xt": np.ascontiguousarray(xt[sl]).astype(ml_dtypes.bfloat16),
            "wca": wca.astype(ml_dtypes.bfloat16),
            "q0r": np.ascontiguousarray(q0r[sl]),
        })

    _CACHED["in_maps"] = in_maps
    res = run_bass_kernel_spmd(nc, in_maps, core_ids=list(range(N_CORES)))
    LAST_RESULTS = res
    xss128 = np.concatenate([r["xss"][:, :, 0] for r in res.results], 0)  # [16, 128]
    xss_dev = xss128[:, 0:33] + xss128[:, 64:97]

    # ---- host epilogue ([16]-row head) ----
    # true weighted sums: add q0sum * sum_l x_aug_l  (the "+1" of elu+1)
    xsum = np.concatenate([x.sum(1), np.full((N, 1), float(L), f32)], 1)   # [16, 33]
    xss_t = xss_dev + q0sum[:, None] * xsum
    xs, ssum = xss_t[:, :32], xss_t[:, 32]

    Z = 1.0 / (ssum + EPS_ATTN)                         # [16]
    hsum = xs @ w_in.T + ssum[:, None] * b_in           # sum_l s_l h_l
    v_att = hsum @ wv.T + ssum[:, None] * bv            # sum_l s_l v_l
    attn_o = (v_att * Z[:, None]) @ wo.T + bo
    t1 = h0 + attn_o
    h1 = _ln(t1, g1, b1)
    y = np.maximum(h1 @ w_ff1.T + b_ff1, 0.0) @ w_ff2.T + b_ff2
    h2 = _ln(h1 + y, g2, b2)
    h3 = _ln(h2, gf, bf)
    out = h3 @ w_fc.T + b_fc                            # [16, 1]
    return out[:, 0].astype(f32)


# revision 25
# speedup vs baseline: 2.2760x; 1.0345x over previous
"""Trainium2 Bass kernel for nn_LinearTransformer_75892072120460.

Math: the reference returns out[:, 0, 0] -- only sequence position 0 of the
final head survives.  Linear attention at query position 0 collapses to
    s_l   = Q0 . (elu(kraw_l) + 1)          (scalar weight per position)
    attn0 = (sum_l s_l h_l) @ wv.T ... / (sum_l s_l + eps)
with kraw = x @ (w_in.T wk.T) + bc, which is the only O(L) dense work.

Device (per core, 2 batches of the 16):  P = x_aug @ Wc_aug  (K=33 matmul,
bias folded via ones-row), then elu(P) = relu(P) + (min(e^P,1) - 1) computed
as one ACT exp pass + dual-op tensor_scalar + split relu, then the weighted
sum over e as PE matmuls with Q0 broadcast to 33 output rows, and a fused
tensor_tensor_reduce producing xs (32 dims) + ssum per chunk.

Host: weight folding, Q0 at position 0 (16x512), and the tiny [16]-row head.
"""

import os
import numpy as np
import ml_dtypes

N, L, IN_DIM, D, E = 16, 4096, 32, 512, 512
EPS_ATTN = 1e-6
EPS_LN = 1e-5
N_CORES = 8
B_PER_CORE = N // N_CORES          # 2
CHUNK = 512                        # rows (l) per chunk
NCHUNK = L // CHUNK                # 8
NJ = 4                             # e-chunks of 128
SPL = 1024                         # relu split point: [0:SPL] on ACT, rest on DVE

_CACHED = {}
LAST_RESULTS = None


def _build_bass(workbufs=3, relu_mod=(0, 8, 0), psp_bufs=3, pss_bufs=2,
                hwdge=True, early_gdots=False, touchers=False, cache=True):
    if cache and "nc" in _CACHED:
        return _CACHED["nc"]
    import concourse.bass as bass
    import concourse.tile as tile
    import concourse.mybir as mybir
    from concourse import bacc

    f32 = mybir.dt.float32
    bf16 = mybir.dt.bfloat16
    AF = mybir.ActivationFunctionType
    OP = mybir.AluOpType

    nc = bacc.Bacc(None, target_bir_lowering=False)
    xt = nc.dram_tensor("xt", [B_PER_CORE, 33, L], bf16, kind="ExternalInput")
    wca = nc.dram_tensor("wca", [33, E], bf16, kind="ExternalInput")
    q0r = nc.dram_tensor("q0r", [B_PER_CORE, 128, 64 * NJ], bf16, kind="ExternalInput")
    xss = nc.dram_tensor("xss", [B_PER_CORE, 128, 1], f32, kind="ExternalOutput")

    HALF = 2 * CHUNK  # two e-chunks per PSUM tile

    with tile.TileContext(nc) as tc:
        with (
            tc.tile_pool(name="const", bufs=1) as const,
            tc.tile_pool(name="work", bufs=workbufs) as work,
            tc.tile_pool(name="accp", bufs=1) as accp,
            tc.tile_pool(name="psP", bufs=psp_bufs, space=bass.MemorySpace.PSUM) as psP,
            tc.tile_pool(name="psS", bufs=pss_bufs, space=bass.MemorySpace.PSUM) as psS,
        ):
            # Wait-absorbers: several instruction structs (fp32 self-loading
            # LDW, STT) accept only ONE sem wait.  A single-tensor read on
            # each engine advances its observed DMA tick first, so the real
            # compute instructions need at most one wait each.
            wc_sb = const.tile([33, E], bf16, tag="wc")
            nc.gpsimd.dma_start(out=wc_sb[:], in_=wca[:])
            nc.tensor.ldweights(wc_sb[:, 0:64], tile_position=(0, 0))

            xbs, q0s, slots = [], [], []
            for n in range(B_PER_CORE):
                xb = const.tile([128, L], bf16, tag=f"xb{n}")
                for c in range(NCHUNK):
                    cs = slice(c * CHUNK, (c + 1) * CHUNK)
                    eng = nc.sync if hwdge else nc.gpsimd
                    eng.dma_start(out=xb[0:33, cs], in_=xt[n][:, cs])
                    eng.dma_start(out=xb[64:97, cs], in_=xt[n][:, cs])
                q0b = const.tile([128, 64 * NJ], bf16, tag=f"q0{n}")
                nc.gpsimd.dma_start(out=q0b[:], in_=q0r[n])
                nc.tensor.ldweights(q0b[:, 0:64], tile_position=(0, 0))
                sl = accp.tile([128, NCHUNK], f32, tag=f"sl{n}")
                xbs.append(xb); q0s.append(q0b); slots.append(sl)

            for n in range(B_PER_CORE):
                xb, q0b, sl = xbs[n], q0s[n], slots[n]
                for c in range(NCHUNK):
                    cs = slice(c * CHUNK, (c + 1) * CHUNK)
                    relu_on_act = (relu_mod[0] * c) % relu_mod[1] < relu_mod[2]
                    # main matmuls into two PSUM half-spans (2 banks each)
                    Ph = []
                    for h in range(2):
                        P_ps = psP.tile([128, HALF], f32, tag="P")
                        for jj in range(2):
                            j = 2 * h + jj
                            for q in range(2):
                                nc.tensor.matmul(
                                    P_ps[64 * q:64 * (q + 1),
                                         jj * CHUNK:(jj + 1) * CHUNK],
                                    wc_sb[:, j * 128 + 64 * q:j * 128 + 64 * (q + 1)],
                                    xb[0:33, cs],
                                    start=True, stop=True,
                                    tile_position=(0, 64 * q),
                                )
                        Ph.append(P_ps)
                    # pass A: E = exp(P)   (ACT, PSUM->SBUF, bf16 out)
                    Eb = work.tile([128, NJ * CHUNK], bf16, tag="E")
                    for h in range(2):
                        nc.scalar.activation(
                            Eb[:, h * HALF:(h + 1) * HALF], Ph[h][:], AF.Exp)
                    # pass B: G = min(E,1) - 1   (DVE 4x bf16, one op)
                    Gb = work.tile([128, NJ * CHUNK], bf16, tag="G")
                    nc.vector.tensor_scalar(
                        Gb[:], Eb[:], 1.0, 1.0, OP.min, OP.subtract)
                    sB = psS.tile([128, CHUNK], f32, tag="sB")

                    def dot(j, src, start, stop):
                        grp = 0 if j < 2 else 1
                        nc.tensor.matmul(
                            sB[64 * grp:64 * (grp + 1), :],
                            q0b[:, 64 * j:64 * j + 64],
                            src[:, j * CHUNK:(j + 1) * CHUNK],
                            start=start, stop=stop,
                            tile_position=(0, 64 * grp),
                        )

                    def gdots():
                        for j in range(NJ):
                            dot(j, Gb, start=(j in (0, 2)), stop=False)

                    if early_gdots:
                        gdots()
                    # pass R: T = relu(P), engine chosen per chunk
                    # (per-engine tag: slot reuse stays same-engine WAW)
                    Tb = work.tile([128, NJ * CHUNK], bf16,
                                   tag="Ta" if relu_on_act else "Tv")
                    for h in range(2):
                        dst = Tb[:, h * HALF:(h + 1) * HALF]
                        if relu_on_act:
                            nc.scalar.activation(dst, Ph[h][:], AF.Relu)
                        else:
                            nc.vector.tensor_scalar(
                                dst, Ph[h][:], 0.0, None, OP.max)
                    # dots: sB[33, l] = sum_e Q0_e * (G + T)    (M=33 broadcast)
                    if not early_gdots:
                        gdots()
                    for j in range(NJ):
                        dot(j, Tb, start=False, stop=(j in (1, 3)))
                    # fused xs/ssum: accum_out[p] = sum_l xb[p, l] * sB[p, l]
                    # (tiny toucher first: absorbs this chunk's DMA tick on
                    # DVE so the STT below only waits on PE)
                    if touchers:
                        nc.vector.tensor_copy(
                            sl[0:1, c:c + 1],
                            xb[0:1, c * CHUNK:c * CHUNK + 1])
                    junk = work.tile([128, CHUNK], f32, tag="junk")
                    nc.vector.scalar_tensor_tensor(
                        out=junk[:],
                        in0=xb[:, cs],
                        scalar=1.0,
                        in1=sB[:],
                        op0=OP.mult,
                        op1=OP.mult,
                        accum_out=sl[:, c:c + 1],
                    )
                # per-batch: reduce the chunk partials, ship out
                xo = accp.tile([128, 1], f32, tag=f"xo{n}")
                nc.vector.tensor_reduce(
                    out=xo[:], in_=sl[:], axis=mybir.AxisListType.X, op=OP.add)
                nc.sync.dma_start(out=xss[n], in_=xo[:])

    nc.compile()
    if cache:
        _CACHED["nc"] = nc
    return nc


def _elu(x):
    return np.where(x > 0, x, np.expm1(np.minimum(x, 0.0)))


def _ln(x, g, b):
    mu = x.mean(-1, keepdims=True)
    var = ((x - mu) ** 2).mean(-1, keepdims=True)
    return (x - mu) / np.sqrt(var + EPS_LN) * g + b


def kernel(x, w_in, b_in, wq, bq, wk, bk, wv, bv, wo, bo, g1, b1,
           w_ff1, b_ff1, w_ff2, b_ff2, g2, b2, gf, bf, w_fc, b_fc):
    global LAST_RESULTS
    from concourse.bass_utils import run_bass_kernel_spmd

    x = np.asarray(x, np.float32)
    f32 = np.float32

    # ---- host weight folding (params only) ----
    Wc = (w_in.T @ wk.T).astype(f32)                    # [32, 512]
    bc = (b_in @ wk.T + bk).astype(f32)                 # [512]
    wca = np.concatenate([Wc, bc[None, :]], 0)          # [33, 512]

    # ---- Q0 at position 0 (host; 16x512, ~0.5 MFLOP) ----
    x0 = x[:, 0, :]                                     # [16, 32]
    h0 = (x0 @ w_in.T + b_in).astype(f32)               # [16, 512]
    q0 = (_elu(h0 @ wq.T + bq) + 1.0).astype(f32)       # [16, 512]
    q0sum = q0.sum(1)                                   # [16]

    # q0 replicated to 64 cols per e-chunk, partition-major
    q0r = np.zeros((N, 128, 64 * NJ), dtype=f32)
    for j in range(NJ):
        q0r[:, :, 64 * j:64 * (j + 1)] = q0[:, 128 * j:128 * (j + 1)][:, :, None]
    q0r = q0r.astype(ml_dtypes.bfloat16)

    # x^T with ones row (bias + ssum row)
    xt = np.concatenate(
        [np.ascontiguousarray(x.transpose(0, 2, 1)),
         np.ones((N, 1, L), f32)], axis=1)              # [16, 33, 4096]

    nc = _build_bass()
    in_maps = []
    for i in range(N_CORES):
        sl = slice(i * B_PER_CORE, (i + 1) * B_PER_CORE)
        in_maps.append({
            "xt": np.ascontiguousarray(xt[sl]).astype(ml_dtypes.bfloat16),
            "wca": wca.astype(ml_dtypes.bfloat16),
            "q0r": np.ascontiguousarray(q0r[sl]),
        })

    _CACHED["in_maps"] = in_maps
    res = run_bass_kernel_spmd(nc, in_maps, core_ids=list(range(N_CORES)))
    LAST_RESULTS = res
    xss128 = np.concatenate([r["xss"][:, :, 0] for r in res.results], 0)  # [16, 128]
    xss_dev = xss128[:, 0:33] + xss128[:, 64:97]

    # ---- host epilogue ([16]-row head) ----
    # true weighted sums: add q0sum * sum_l x_aug_l  (the "+1" of elu+1)
    xsum = np.concatenate([x.sum(1), np.full((N, 1), float(L), f32)], 1)   # [16, 33]
    xss_t = xss_dev + q0sum[:, None] * xsum
    xs, ssum = xss_t[:, :32], xss_t[:, 32]

    Z = 1.0 / (ssum + EPS_ATTN)                         # [16]
    hsum = xs @ w_in.T + ssum[:, None] * b_in           # sum_l s_l h_l
    v_att = hsum @ wv.T + ssum[:, None] * bv            # sum_l s_l v_l
    attn_o = (v_att * Z[:, None]) @ wo.T + bo
    t1 = h0 + attn_o
    h1 = _ln(t1, g1, b1)
    y = np.maximum(h1 @ w_ff1.T + b_ff1, 0.0) @ w_ff2.T + b_ff2
    h2 = _ln(h1 + y, g2, b2)
    h3 = _ln(h2, gf, bf)
    out = h3 @ w_fc.T + b_fc                            # [16, 1]
    return out[:, 0].astype(f32)
